# revision 31
# baseline (speedup 1.0000x reference)
"""Trainium2 kernel for the 101-layer scalar-affine+ReLU chain.

The reference applies h -> relu(w_i * h + b_i) for i = 0..100 elementwise on a
(32, 1, 1024, 1024) f32 tensor. Each step is x -> max(0, w*x + b); for w >= 0
the composition of such maps stays in the closed form

    F(x) = max(C, A*x + D)

with the recursion  C' = max(0, w*C + b),  A' = w*A,  D' = w*D + b  (start
C = -inf, A = 1, D = 0).  So the whole chain is one clamp-affine, and the
kernel is a single memory-bound elementwise pass:

    out = relu(A*x + (D - C)) + C

The pass is HBM-bound (358 GB/s per core), so I/O precision is traded for
bandwidth within the 2e-2 rel-err budget: the host quantizes x to fp16
(error ~2^-11), and the device computes the clamp-affine and writes the
result log-quantized to uint8 (q = round(ln(out/C)/s), a 256-level
geometric grid -> half-step rel err ~0.9%), which the host decodes with a
256-entry LUT. Per-core HBM traffic drops from 32 MiB (f32) to 12 MiB.

Device pipeline per tile: ACT Ln with its free affine (u = ln((A/C)x + D/C),
one pass; clamped inputs give z <= 0 -> NaN/-inf) -> DVE quantize
(q = max(u/s, 0) as u8; the max suppresses the NaN to exactly q=0 == out=C,
verified on HW). Loads issue on the SP HWDGE ring, stores on the ACT ring
so they don't queue behind pending loads.

Sharding: pure data parallel, batch 32 split 4-per-core across 8 cores.
_plan() simulates the pipeline's error on a host subsample and falls back
to fp16 or f32 I/O if the (w, b) at hand ever made u8 too coarse.

Perf notes (bench.py, device For_i slope timing; ~±1 us run-to-run drift):
the pass is pinned to the mixed read+write DMA wall — a pure-DMA probe of
the same traffic (no compute at all) runs no faster than the full kernel,
loads alone sustain ~330 GB/s, u8 stores ~292 GB/s, and a 1-core run is
only ~15% faster (chip-level HBM contention). Structural rewrites that
looked promising all lost to the baseline shape: 8K-wide tiles (bigger
DMAs, but late ACT start + heavy drain), merged stores, and especially
splitting the per-core shard across two dram tensors (consistently 3-6 us
slower even in pure-DMA probes — single contiguous allocation matters).
What did survive A/B: store issue lagging compute by 2 chunks plus
head/tail column-splits of the first/last tile (PROD_KW), worth ~0.5-1 us.
"""

import numpy as np

N_CORES = 8
FULL_SHAPE = (32, 1, 1024, 1024)
PER_CORE_ELEMS = (FULL_SHAPE[0] // N_CORES) * FULL_SHAPE[1] * FULL_SHAPE[2] * FULL_SHAPE[3]

P = 128          # SBUF partitions
FREE = 4096      # free-dim elements per tile  (fp16 tile = 128*4096*2B = 1 MiB)
NT = PER_CORE_ELEMS // (P * FREE)  # tiles per core

_nc_cache = {}


def _collapse(w, b):
    """Fold the relu-affine chain into (A, D, C) with F(x) = max(C, A*x + D)."""
    a = np.float64(1.0)
    d = np.float64(0.0)
    c = -np.inf
    for wi, bi in zip(w.astype(np.float64), b.astype(np.float64)):
        c = max(0.0, float(wi * c + bi))
        a = wi * a
        d = wi * d + bi
    return float(a), float(d), float(c)


def _build(A, D, C, iters=None, free=FREE, bufs=4, dma_only=False,
           mode="f16", s=None, quant_round=True, probe=None, tail_split=None,
           chunk=None, pipe=0, head_split=None):
    """Build the bass program. iters=None -> single pass (the real kernel);
    iters=k -> the same pass wrapped in a device-side For_i loop, used only
    by the timing harness (slope over k cancels host/RPC overhead).

    mode="f16": fp16 in -> max(C, A*x+D) -> fp16 out.
    mode="u8":  fp16 in -> q = round(ln(max(C, A*x+D)/C)/s) -> uint8 out
                (log-quantized output, decoded on host via a 256-entry LUT;
                halves the store-side HBM traffic)."""
    import concourse.bacc as bacc
    import concourse.mybir as mybir
    from concourse.tile import TileContext

    nt = PER_CORE_ELEMS // (P * free)
    # Bacc (not raw Bass): its finalize() runs generate_event_semaphores,
    # which splits multi-sem waits to satisfy TRN2's 1-wait-per-instruction
    # hardware constraint.
    nc = bacc.Bacc("TRN2", target_bir_lowering=False)
    in_dt = mybir.dt.float32 if mode == "f32" else mybir.dt.float16
    out_dt = {"u8": mybir.dt.uint8, "f32": mybir.dt.float32}.get(mode, mybir.dt.float16)
    x = nc.dram_tensor("x", [nt * P, free], in_dt, kind="ExternalInput")
    y = nc.dram_tensor("y", [nt * P, free], out_dt, kind="ExternalOutput")
    relu = mybir.ActivationFunctionType.Relu
    ln_f = mybir.ActivationFunctionType.Ln

    # Materialize the ACT bias constant outside the Tile program, behind a
    # barrier (same pattern Bass.__init__ uses for its 0.0/1.0 const APs), so
    # the Activation instructions don't pick up an extra sync wait.
    bias_tensor = nc.alloc_sbuf_tensor("bias_dc", [P, 1], mybir.dt.float32)
    bias_val = float(D / C) if mode == "u8" else float(D - C)
    nc.gpsimd.memset(bias_tensor.ap(), bias_val)
    # Tiny pre-loop Ln so the ACT table set loads once outside the For_i body.
    warm = nc.alloc_sbuf_tensor("warm", [P, 1], mybir.dt.float32)
    if mode == "u8":
        nc.gpsimd.memset(warm.ap(), 1.0)
    nc.all_engine_barrier()
    if mode == "u8":
        nc.scalar.activation(warm.ap(), warm.ap(), ln_f, bias=bias_tensor.ap()[:, :1],
                             scale=float(A / C))
        nc.all_engine_barrier()
    bias_t = bias_tensor.ap()

    x0 = (C - D) / A  # clamp threshold: max(C, A*x+D) == A*max(x, x0) + D

    scratch_q = None
    if probe in ("dmaonly3", "dma3sc", "dma3lda", "noquant", "stonly", "nodve"):
        scratch_q = nc.alloc_sbuf_tensor("scratch_q", [P, free], out_dt)
        nc.gpsimd.memset(scratch_q.ap(), 0)
        nc.all_engine_barrier()
    scratch_u = None
    if probe in ("dveonly", "actonly", "gponly"):
        scratch_u = nc.alloc_sbuf_tensor("scratch_u", [P, free], mybir.dt.float16)
        nc.gpsimd.memset(scratch_u.ap(), 1.0)
        nc.all_engine_barrier()

    if isinstance(bufs, int):
        bufs = (bufs, bufs, bufs)

    with TileContext(nc) as tc:
        with (
            tc.tile_pool(name="ld", bufs=bufs[0]) as ld_pool,
            tc.tile_pool(name="mid", bufs=bufs[1]) as mid_pool,
            tc.tile_pool(name="st", bufs=bufs[2]) as st_pool,
        ):
            # u8 stores issue on the ACT HWDGE ring so they don't queue
            # behind pending loads in the SP ring (measured ~5us/pass win).
            st_eng = (nc.scalar if (mode == "u8" and probe is None)
                      or probe in ("stsc", "dma3sc") else nc.sync)

            def ld_eng_for(i):
                if probe in ("lda", "3ring", "dma3lda"):
                    return nc.sync if i % 2 == 0 else nc.scalar
                if probe == "swap":
                    return nc.scalar
                return nc.sync

            def st_eng_for(i):
                if probe == "swap":
                    return nc.sync
                if probe in ("lda", "dma3lda"):
                    return nc.scalar if i % 2 == 0 else nc.sync
                if probe in ("3ring", "gpst"):
                    return nc.gpsimd
                if probe == "stsp":
                    return nc.sync
                return st_eng

            def one_pass_pipe():
                # Software-pipelined u8 pass: the store of chunk k is issued
                # on the ACT ring only after the Ln of chunk k+pipe has been
                # issued, so the ACT sequencer never blocks in a store
                # dispatch waiting for the DVE quantize semaphore.
                pend = []

                def emit(i, c0, wdt, t=None):
                    pend.append(u8_chain(i, c0, wdt, t, defer_store=True))
                    while len(pend) > pipe:
                        j, d0, dw, q = pend.pop(0)
                        st_eng_for(j).dma_start(y[j * P:(j + 1) * P, d0:d0 + dw], q[:])

                for i in range(nt):
                    if head_split and i == 0:
                        # Small separate sub-loads so the first Ln starts as
                        # early as possible.
                        c0 = 0
                        for wdt in head_split:
                            emit(i, c0, wdt)
                            c0 += wdt
                        assert c0 == free, (c0, free)
                    elif tail_split and i == nt - 1:
                        # Small sub-loads and small stores: the last chunk's
                        # load lands early and its compute+store chain is
                        # short, so the pipeline drains quickly.
                        c0 = 0
                        for wdt in tail_split:
                            emit(i, c0, wdt)
                            c0 += wdt
                        assert c0 == free, (c0, free)
                    elif chunk:
                        t = ld_pool.tile([P, free], in_dt)
                        ld_eng_for(i).dma_start(t[:], x[i * P:(i + 1) * P, :])
                        for c0 in range(0, free, chunk):
                            emit(i, c0, chunk, t)
                    else:
                        emit(i, 0, free)
                for j, d0, dw, q in pend:
                    st_eng_for(j).dma_start(y[j * P:(j + 1) * P, d0:d0 + dw], q[:])

            def one_pass():
                for i in range(nt):
                    if probe == "ldonly":
                        t = ld_pool.tile([P, free], in_dt)
                        nc.sync.dma_start(t[:], x[i * P:(i + 1) * P, :])
                        continue
                    if probe == "stonly":
                        nc.scalar.dma_start(y[i * P:(i + 1) * P, :],
                                            scratch_q.ap())
                        continue
                    if probe == "st2r":
                        # each store split in column halves across both rings
                        h = free // 2
                        nc.scalar.dma_start(y[i * P:(i + 1) * P, :h],
                                            scratch_q.ap()[:, :h])
                        nc.sync.dma_start(y[i * P:(i + 1) * P, h:],
                                          scratch_q.ap()[:, h:])
                        continue
                    if probe == "dveonly":
                        q = st_pool.tile([P, free], mybir.dt.uint8)
                        nc.vector.tensor_scalar(q[:], scratch_u.ap(),
                                                float(1.0 / s), 0.0,
                                                mybir.AluOpType.mult,
                                                mybir.AluOpType.max)
                        continue
                    if probe == "gponly":
                        q = st_pool.tile([P, free], mybir.dt.uint8)
                        nc.gpsimd.tensor_scalar(q[:], scratch_u.ap(),
                                                float(1.0 / s), 0.0,
                                                mybir.AluOpType.mult,
                                                mybir.AluOpType.max)
                        continue
                    if probe == "actonly":
                        u = mid_pool.tile([P, free], mybir.dt.float16)
                        nc.scalar.activation(u[:], scratch_u.ap(), ln_f,
                                             bias=bias_t[:, :1],
                                             scale=float(A / C))
                        continue
                    if probe == "nodve":
                        t = ld_pool.tile([P, free], in_dt)
                        nc.sync.dma_start(t[:], x[i * P:(i + 1) * P, :])
                        u = mid_pool.tile([P, free], mybir.dt.float16)
                        nc.scalar.activation(u[:], t[:], ln_f,
                                             bias=bias_t[:, :1],
                                             scale=float(A / C))
                        nc.scalar.dma_start(y[i * P:(i + 1) * P, :],
                                            scratch_q.ap())
                        continue
                    if (mode == "u8" and tail_split and i == nt - 1
                            and probe in (None, "stsc")):
                        c0 = 0
                        for wdt in tail_split:
                            u8_chain(i, c0, wdt)
                            c0 += wdt
                        assert c0 == free, (c0, free)
                        continue
                    if (mode == "u8" and chunk and probe in (None, "stsc")):
                        t = ld_pool.tile([P, free], in_dt)
                        nc.sync.dma_start(t[:], x[i * P:(i + 1) * P, :])
                        for c0 in range(0, free, chunk):
                            u8_chain(i, c0, chunk, t)
                        continue
                    t = ld_pool.tile([P, free], in_dt)
                    ld_eng_for(i).dma_start(t[:], x[i * P:(i + 1) * P, :])
                    if dma_only:
                        nc.sync.dma_start(y[i * P:(i + 1) * P, :], t[:])
                        continue
                    if mode in ("f16", "f32"):
                        o = st_pool.tile([P, free], in_dt)
                        # o = relu(A*x + (D - C))
                        nc.scalar.activation(o[:], t[:], relu, bias=bias_t[:, :1],
                                             scale=float(A))
                        # o += C  ->  o = max(C, A*x + D)
                        nc.vector.tensor_scalar_add(o[:], o[:], float(C))
                        nc.sync.dma_start(y[i * P:(i + 1) * P, :], o[:])
                    else:
                        if probe in ("dmaonly3", "dma3sc", "dma3lda"):
                            st_eng_for(i).dma_start(y[i * P:(i + 1) * P, :],
                                                    scratch_q.ap())
                            continue
                        if probe == "nomax":
                            u = mid_pool.tile([P, free], mybir.dt.float16)
                            nc.scalar.activation(u[:], t[:], ln_f, bias=bias_t[:, :1],
                                                 scale=float(A / C))
                            q = st_pool.tile([P, free], mybir.dt.uint8)
                            nc.vector.tensor_scalar(q[:], u[:], float(1.0 / s), 0.0,
                                                    mybir.AluOpType.mult,
                                                    mybir.AluOpType.max)
                            nc.scalar.dma_start(y[i * P:(i + 1) * P, :], q[:])
                            continue
                        if probe == "noact":
                            nc.vector.tensor_scalar_max(t[:], t[:], float(x0))
                            q = st_pool.tile([P, free], mybir.dt.uint8)
                            nc.vector.tensor_scalar(q[:], t[:], float(1.0 / s), 0.0,
                                                    mybir.AluOpType.mult,
                                                    mybir.AluOpType.max)
                            nc.sync.dma_start(y[i * P:(i + 1) * P, :], q[:])
                            continue
                        u8_chain(i, 0, free, t)

            def u8_chain(i, c0, w, t=None, defer_store=False):
                """One load->max->ln->quantize->store chain on columns
                [c0, c0+w) of row-block i. t: already-loaded [P, free] block
                tile (compute on its [:, c0:c0+w] slice) or None (load)."""
                if t is None:
                    tc_tile = ld_pool.tile([P, w], in_dt)
                    ts = tc_tile[:]
                    nc.sync.dma_start(ts, x[i * P:(i + 1) * P, c0:c0 + w])
                else:
                    ts = t[:, c0:c0 + w]
                # u = ln((A/C)*x + D/C) = ln(out/C), no pre-clamp: for clamped
                # inputs z <= 0, Ln yields NaN/-inf and the quantize op's
                # trailing max() suppresses it to exactly q=0 == out=C
                # (verified on HW over the 18M clamped elements).
                u = mid_pool.tile([P, w], mybir.dt.float16)
                nc.scalar.activation(u[:], ts, ln_f, bias=bias_t[:, :1],
                                     scale=float(A / C))
                if probe == "noquant":
                    nc.sync.dma_start(y[i * P:(i + 1) * P, c0:c0 + w],
                                      scratch_q.ap()[:, c0:c0 + w])
                    return
                q = st_pool.tile([P, w], mybir.dt.uint8)
                qeng = nc.gpsimd if (probe == "gq" or
                                     (probe == "alt" and i % 2)) else nc.vector
                if quant_round:
                    # fp->u8 convert rounds to nearest (verified on HW)
                    qeng.tensor_scalar(q[:], u[:], float(1.0 / s), 0.0,
                                       mybir.AluOpType.mult,
                                       mybir.AluOpType.max)
                else:
                    qeng.tensor_scalar(q[:], u[:], float(1.0 / s), 0.5,
                                       mybir.AluOpType.mult,
                                       mybir.AluOpType.add)
                if defer_store:
                    return (i, c0, w, q)
                st_eng_for(i).dma_start(y[i * P:(i + 1) * P, c0:c0 + w], q[:])

            body = (one_pass_pipe if (mode == "u8" and (pipe or head_split)
                                      and probe in (None, "stsc", "gpst", "stsp"))
                    else one_pass)
            if iters is None:
                body()
            else:
                with tc.For_i(0, iters, 1):
                    body()
    nc.finalize()
    return nc


# Production build parameters for the u8 kernel (selected by bench.py A/Bs):
# store issue lags compute by 2 chunks (the ACT sequencer never blocks in a
# store dispatch waiting on the DVE quantize), the first tile is loaded and
# computed in two 2048-column halves (earlier ACT start), and the last tile
# in four 1024-column chunks (short pipeline drain).
PROD_KW = dict(bufs=(12, 5, 6), pipe=2, head_split=(2048, 2048),
               tail_split=(1024, 1024, 1024, 1024))

# (Dead end, kept for the record: splitting the per-core shard across two
# dram tensors — to allow non-uniform tile sizes / merged stores — was
# consistently 3-6 us SLOWER than the single-tensor layout, even with
# identical tile geometry and even in pure-DMA probes. See _build_v2.)
V2_CFG = dict(tail_free=4096, n_tail=2, main_free=8192, n_main=3)
V2_KW = dict(pipe=2, chunk=4096, head_chunks=(2048, 2048),
             tail_chunks=(1024, 1024, 1024, 1024))


def _build_v2(A, D, C, s, iters=None, probe=None, pipe=2, chunk=4096,
              tail_free=None, n_tail=None, main_free=None, n_main=None,
              head_chunks=None, tail_chunks=(1024, 1024)):
    """Non-uniform tile schedule for the u8 log-quant kernel.

    Per-core input is split into two dram tensors:
      x_tail [n_tail*128, tail_free] fp16 — tail block 0 opens the pipeline
        (small first load -> early ACT start, optionally compute-chunked via
        head_chunks), the remaining tail blocks close it (the last one
        compute-chunked via tail_chunks for a short drain).
      x_main [n_main*128, main_free] fp16 — large mid-stream DMAs for mix
        efficiency.
    Outputs y_tail / y_main u8 mirror the input layout. Main tiles compute
    Ln in `chunk`-wide pieces, quantize into one merged [128, main_free] u8
    buffer, and store it as a single DMA. Store issue on the ACT ring lags
    compute by `pipe` entries so the ACT sequencer never blocks on the DVE
    quantize semaphore.
    """
    import concourse.bacc as bacc
    import concourse.mybir as mybir
    from concourse.tile import TileContext

    tail_free = V2_CFG["tail_free"] if tail_free is None else tail_free
    n_tail = V2_CFG["n_tail"] if n_tail is None else n_tail
    main_free = V2_CFG["main_free"] if main_free is None else main_free
    n_main = V2_CFG["n_main"] if n_main is None else n_main
    assert n_tail * P * tail_free + n_main * P * main_free == PER_CORE_ELEMS
    assert sum(tail_chunks) == tail_free
    assert head_chunks is None or sum(head_chunks) == tail_free

    nc = bacc.Bacc("TRN2", target_bir_lowering=False)
    f16 = mybir.dt.float16
    u8 = mybir.dt.uint8
    xt = nc.dram_tensor("x_tail", [n_tail * P, tail_free], f16,
                        kind="ExternalInput")
    xm = nc.dram_tensor("x_main", [n_main * P, main_free], f16,
                        kind="ExternalInput")
    yt = nc.dram_tensor("y_tail", [n_tail * P, tail_free], u8,
                        kind="ExternalOutput")
    ym = nc.dram_tensor("y_main", [n_main * P, main_free], u8,
                        kind="ExternalOutput")
    ln_f = mybir.ActivationFunctionType.Ln

    bias_tensor = nc.alloc_sbuf_tensor("bias_dc", [P, 1], mybir.dt.float32)
    nc.gpsimd.memset(bias_tensor.ap(), float(D / C))
    warm = nc.alloc_sbuf_tensor("warm", [P, 1], mybir.dt.float32)
    nc.gpsimd.memset(warm.ap(), 1.0)
    scratch = None
    if probe == "dma":
        scratch = nc.alloc_sbuf_tensor("scratch_q", [P, main_free], u8)
        nc.gpsimd.memset(scratch.ap(), 0)
    nc.all_engine_barrier()
    nc.scalar.activation(warm.ap(), warm.ap(), ln_f, bias=bias_tensor.ap()[:, :1],
                         scale=float(A / C))
    nc.all_engine_barrier()
    bias_t = bias_tensor.ap()

    with TileContext(nc) as tc:
        with (
            tc.tile_pool(name="lds", bufs=4) as lds_pool,
            tc.tile_pool(name="ldb", bufs=3) as ldb_pool,
            tc.tile_pool(name="mid", bufs=4) as mid_pool,
            tc.tile_pool(name="stb", bufs=2) as stb_pool,
            tc.tile_pool(name="sts", bufs=4) as sts_pool,
        ):
            def one_pass():
                # ---- issue every load up front on the SP ring ----
                t_tail0 = lds_pool.tile([P, tail_free], f16)
                nc.sync.dma_start(t_tail0[:], xt[0:P, :])
                t_mains = []
                for i in range(n_main):
                    t = ldb_pool.tile([P, main_free], f16)
                    nc.sync.dma_start(t[:], xm[i * P:(i + 1) * P, :])
                    t_mains.append(t)
                t_tails = []
                for i in range(1, n_tail):
                    t = lds_pool.tile([P, tail_free], f16)
                    nc.sync.dma_start(t[:], xt[i * P:(i + 1) * P, :])
                    t_tails.append(t)

                pend = []

                def flush(n_keep):
                    while len(pend) > n_keep:
                        dst, r0, c0, w, q = pend.pop(0)
                        nc.scalar.dma_start(dst[r0:r0 + P, c0:c0 + w], q)

                def compute(t, c0, w, qbuf=None, qoff=0):
                    """ln+quantize columns [c0, c0+w) of loaded tile t into
                    qbuf[:, qoff:qoff+w] (or a fresh small tile)."""
                    u = mid_pool.tile([P, w], f16)
                    nc.scalar.activation(u[:], t[:, c0:c0 + w], ln_f,
                                         bias=bias_t[:, :1], scale=float(A / C))
                    if qbuf is None:
                        qbuf = sts_pool.tile([P, w], u8)
                        qoff = 0
                    nc.vector.tensor_scalar(qbuf[:, qoff:qoff + w], u[:],
                                            float(1.0 / s), 0.0,
                                            mybir.AluOpType.mult,
                                            mybir.AluOpType.max)
                    return qbuf[:, qoff:qoff + w]

                if probe == "dma":
                    # stores immediately from scratch: pure-DMA floor probe
                    nc.scalar.dma_start(yt[0:P, :], scratch.ap()[:, :tail_free])
                    for i in range(n_main):
                        nc.scalar.dma_start(ym[i * P:(i + 1) * P, :], scratch.ap())
                    for i in range(1, n_tail):
                        nc.scalar.dma_start(yt[i * P:(i + 1) * P, :],
                                            scratch.ap()[:, :tail_free])
                    return

                # ---- tail block 0: opener ----
                for c0, w in _chunks(head_chunks or (tail_free,)):
                    q = compute(t_tail0, c0, w)
                    pend.append((yt, 0, c0, w, q))
                    flush(pipe)
                # ---- main blocks: chunked compute, merged store ----
                for i in range(n_main):
                    qbuf = stb_pool.tile([P, main_free], u8)
                    for c0 in range(0, main_free, chunk):
                        compute(t_mains[i], c0, chunk, qbuf, c0)
                        flush(pipe)
                    pend.append((ym, i * P, 0, main_free, qbuf[:]))
                    flush(pipe)
                # ---- remaining tail blocks: closers ----
                for bi in range(1, n_tail):
                    t = t_tails[bi - 1]
                    widths = tail_chunks if bi == n_tail - 1 else (tail_free,)
                    for c0, w in _chunks(widths):
                        q = compute(t, c0, w)
                        pend.append((yt, bi * P, c0, w, q))
                        flush(pipe)
                flush(0)

            if iters is None:
                one_pass()
            else:
                with tc.For_i(0, iters, 1):
                    one_pass()
    nc.finalize()
    return nc


def _chunks(widths):
    c0 = 0
    out = []
    for w in widths:
        out.append((c0, w))
        c0 += w
    return out


def _make_shards_v2(x_f32, cfg=None):
    """Full f32 (32,1,1024,1024) -> per-core {x_tail, x_main} fp16 shards."""
    cfg = cfg or V2_CFG
    xh = np.ascontiguousarray(x_f32, dtype=np.float32).astype(np.float16)
    flat = xh.reshape(N_CORES, PER_CORE_ELEMS)
    nt0 = P * cfg["tail_free"]                    # tail block 0
    nm = cfg["n_main"] * P * cfg["main_free"]     # main blocks
    maps = []
    for k in range(N_CORES):
        f = flat[k]
        tail = np.concatenate([f[:nt0], f[nt0 + nm:]])
        maps.append({
            "x_tail": np.ascontiguousarray(
                tail.reshape(cfg["n_tail"] * P, cfg["tail_free"])),
            "x_main": np.ascontiguousarray(
                f[nt0:nt0 + nm].reshape(cfg["n_main"] * P, cfg["main_free"])),
        })
    return maps


def _decode_v2(results, C, s, cfg=None):
    """Per-core {y_tail, y_main} u8 -> full f32 output."""
    cfg = cfg or V2_CFG
    lut = (np.float64(C) * np.exp(np.float64(s) * np.arange(256))).astype(np.float32)
    nt0 = P * cfg["tail_free"]
    nm = cfg["n_main"] * P * cfg["main_free"]
    outs = []
    for res in results:
        tail = np.asarray(res["y_tail"]).reshape(-1)
        main = np.asarray(res["y_main"]).reshape(-1)
        q = np.empty(PER_CORE_ELEMS, np.uint8)
        q[:nt0] = tail[:nt0]
        q[nt0:nt0 + nm] = main
        q[nt0 + nm:] = tail[nt0:]
        outs.append(lut[q].reshape(FULL_SHAPE[0] // N_CORES, *FULL_SHAPE[1:]))
    return np.concatenate(outs, axis=0)


def _make_shards(x_f32, mode="u8"):
    """Full f32 (32,1,1024,1024) -> 8 contiguous per-core shards [NT*P, FREE]."""
    dt = np.float32 if mode == "f32" else np.float16
    xh = np.ascontiguousarray(x_f32, dtype=np.float32).astype(dt)
    shards = xh.reshape(N_CORES, NT * P, FREE)
    return [np.ascontiguousarray(shards[k]) for k in range(N_CORES)]


def _simulate_rel_err(xh, A, D, C, mode, s=None):
    """Max rel-err of the device pipeline (simulated on host) vs the exact
    collapsed map, on a subsample. xh: fp16 input subsample."""
    xs = xh.astype(np.float32)
    exact = np.maximum(C, np.float64(A) * xh.astype(np.float64) + np.float64(D))
    if mode == "f16":
        t = np.maximum(np.float32(A) * xs + np.float32(D - C),
                       np.float32(0)).astype(np.float16)
        out = (t.astype(np.float32) + np.float32(C)).astype(np.float16).astype(np.float64)
    elif mode == "u8":
        x0 = (C - D) / A
        m = np.maximum(xs, np.float32(x0))
        z = np.float32(A / C) * m + np.float32(D / C)
        u = np.log(np.maximum(z, np.float32(1e-37))).astype(np.float16)
        q = np.clip(np.rint(np.maximum(
            u.astype(np.float32) * np.float32(1.0 / s), np.float32(0.0))),
            0, 255).astype(np.uint8)
        out = np.float64(C) * np.exp(np.float64(s) * q.astype(np.float64))
    else:
        return 0.0
    denom = np.maximum(np.abs(exact), 1e-6)
    return float((np.abs(out - exact) / denom).max())


def _plan(x, w, b):
    """Choose the device pipeline (u8 / f16 / f32) and its parameters."""
    A, D, C = _collapse(w, b)
    xh = x.astype(np.float16)
    sample = xh.reshape(-1)[::5]
    out_max = float(A) * float(xh.max()) + float(D)
    if C > 1e-30 and out_max > C:
        s = float(np.log(out_max * 1.001 / C) / 255.0)
        if _simulate_rel_err(sample, A, D, C, "u8", s) < 1.5e-2:
            return A, D, C, "u8", s
    if _simulate_rel_err(sample, A, D, C, "f16") < 1.5e-2:
        return A, D, C, "f16", None
    return A, D, C, "f32", None


def _run_device(x, A, D, C, mode, s, trace=False):
    from concourse.bass_utils import run_bass_kernel_spmd

    key = (mode, round(A, 12), round(D, 12), round(C, 12),
           None if s is None else round(s, 14))
    nc = _nc_cache.get(key)
    if nc is None:
        if mode == "u8":
            nc = _build(A, D, C, mode=mode, s=s, **PROD_KW)
        else:
            nc = _build(A, D, C, mode=mode, s=s, bufs=4)
        _nc_cache[key] = nc

    in_maps = [{"x": sh} for sh in _make_shards(x, mode)]
    try:
        res = run_bass_kernel_spmd(nc, in_maps, list(range(N_CORES)), trace=trace)
    except Exception:
        # The axon-tunneled devices occasionally come up wedged from a prior
        # interrupted session (NRT_EXEC_UNIT_UNRECOVERABLE); one retry after a
        # short pause reliably recovers.
        import time
        time.sleep(15)
        res = run_bass_kernel_spmd(nc, in_maps, list(range(N_CORES)), trace=trace)
    ys = [res.results[k]["y"] for k in range(N_CORES)]
    if mode == "u8":
        lut = (np.float64(C) * np.exp(np.float64(s) * np.arange(256))).astype(np.float32)
        ys = [lut[yk] for yk in ys]
    out = np.concatenate(
        [yk.astype(np.float32).reshape(FULL_SHAPE[0] // N_CORES, *FULL_SHAPE[1:])
         for yk in ys],
        axis=0,
    )
    return out, res


def kernel(x, w, b, trace=False, _return_res=False):
    x = np.ascontiguousarray(np.asarray(x, dtype=np.float32))
    w = np.asarray(w, dtype=np.float32)
    b = np.asarray(b, dtype=np.float32)
    assert x.shape == FULL_SHAPE, x.shape

    if np.any(w < 0.0):
        # Not reachable for the given distribution (w ~ N(1, 0.02^2)); exact
        # host fallback to keep the kernel correct for arbitrary params.
        h = x.copy()
        for wi, bi in zip(w, b):
            h = np.maximum(h * wi + bi, np.float32(0.0)).astype(np.float32)
        return h

    A, D, C, mode, s = _plan(x, w, b)
    out, res = _run_device(x, A, D, C, mode, s, trace=trace)
    out = out.astype(np.float32, copy=False)
    if _return_res:
        return out, res
    return out



# revision 39
# speedup vs baseline: 1.0094x; 1.0094x over previous
"""Trainium2 kernel for the 101-layer scalar-affine+ReLU chain.

The reference applies h -> relu(w_i * h + b_i) for i = 0..100 elementwise on a
(32, 1, 1024, 1024) f32 tensor. Each step is x -> max(0, w*x + b); for w >= 0
the composition of such maps stays in the closed form

    F(x) = max(C, A*x + D)

with the recursion  C' = max(0, w*C + b),  A' = w*A,  D' = w*D + b  (start
C = -inf, A = 1, D = 0).  So the whole chain is one clamp-affine, and the
kernel is a single memory-bound elementwise pass:

    out = relu(A*x + (D - C)) + C

The pass is HBM-bound (358 GB/s per core), so I/O precision is traded for
bandwidth within the 2e-2 rel-err budget: the host quantizes x to fp16
(error ~2^-11), and the device computes the clamp-affine and writes the
result log-quantized to uint8 (q = round(ln(out/C)/s), a 256-level
geometric grid -> half-step rel err ~0.9%), which the host decodes with a
256-entry LUT. Per-core HBM traffic drops from 32 MiB (f32) to 12 MiB.

Device pipeline per tile: ACT Ln with its free affine (u = ln((A/C)x + D/C),
one pass; clamped inputs give z <= 0 -> NaN/-inf) -> DVE quantize
(q = max(u/s, 0) as u8; the max suppresses the NaN to exactly q=0 == out=C,
verified on HW). Loads issue on the SP HWDGE ring, stores on the ACT ring
so they don't queue behind pending loads.

Sharding: pure data parallel, batch 32 split 4-per-core across 8 cores.
_plan() simulates the pipeline's error on a host subsample and falls back
to fp16 or f32 I/O if the (w, b) at hand ever made u8 too coarse.

Perf notes (bench.py, device For_i slope timing; ~±1 us run-to-run drift):
the pass is pinned to the mixed read+write DMA wall — a pure-DMA probe of
the same traffic (no compute at all) runs no faster than the full kernel,
loads alone sustain ~330 GB/s, u8 stores ~292 GB/s, and a 1-core run is
only ~15% faster (chip-level HBM contention). Structural rewrites that
looked promising all lost to the baseline shape: 8K-wide tiles (bigger
DMAs, but late ACT start + heavy drain), merged stores, and especially
splitting the per-core shard across two dram tensors (consistently 3-6 us
slower even in pure-DMA probes — single contiguous allocation matters).
What did survive A/B: store issue lagging compute by 2 chunks plus
head/tail column-splits of the first/last tile (PROD_KW), worth ~0.5-1 us.
"""

import numpy as np

N_CORES = 8
FULL_SHAPE = (32, 1, 1024, 1024)
PER_CORE_ELEMS = (FULL_SHAPE[0] // N_CORES) * FULL_SHAPE[1] * FULL_SHAPE[2] * FULL_SHAPE[3]

P = 128          # SBUF partitions
FREE = 4096      # free-dim elements per tile  (fp16 tile = 128*4096*2B = 1 MiB)
NT = PER_CORE_ELEMS // (P * FREE)  # tiles per core

_nc_cache = {}


def _collapse(w, b):
    """Fold the relu-affine chain into (A, D, C) with F(x) = max(C, A*x + D)."""
    a = np.float64(1.0)
    d = np.float64(0.0)
    c = -np.inf
    for wi, bi in zip(w.astype(np.float64), b.astype(np.float64)):
        c = max(0.0, float(wi * c + bi))
        a = wi * a
        d = wi * d + bi
    return float(a), float(d), float(c)


def _build(A, D, C, iters=None, free=FREE, bufs=4, dma_only=False,
           mode="f16", s=None, quant_round=True, probe=None, tail_split=None,
           chunk=None, pipe=0, head_split=None, split_st_from=None):
    # split_st_from: tile index from which stores are issued as two
    # column-half DMAs, one per HWDGE ring — by then the SP ring has drained
    # its loads, so both rings carry store packets in parallel during the
    # drain phase.
    """Build the bass program. iters=None -> single pass (the real kernel);
    iters=k -> the same pass wrapped in a device-side For_i loop, used only
    by the timing harness (slope over k cancels host/RPC overhead).

    mode="f16": fp16 in -> max(C, A*x+D) -> fp16 out.
    mode="u8":  fp16 in -> q = round(ln(max(C, A*x+D)/C)/s) -> uint8 out
                (log-quantized output, decoded on host via a 256-entry LUT;
                halves the store-side HBM traffic)."""
    import concourse.bacc as bacc
    import concourse.mybir as mybir
    from concourse.tile import TileContext

    nt = PER_CORE_ELEMS // (P * free)
    # Bacc (not raw Bass): its finalize() runs generate_event_semaphores,
    # which splits multi-sem waits to satisfy TRN2's 1-wait-per-instruction
    # hardware constraint.
    nc = bacc.Bacc("TRN2", target_bir_lowering=False)
    in_dt = mybir.dt.float32 if mode == "f32" else mybir.dt.float16
    out_dt = {"u8": mybir.dt.uint8, "f32": mybir.dt.float32}.get(mode, mybir.dt.float16)
    x = nc.dram_tensor("x", [nt * P, free], in_dt, kind="ExternalInput")
    y = nc.dram_tensor("y", [nt * P, free], out_dt, kind="ExternalOutput")
    relu = mybir.ActivationFunctionType.Relu
    ln_f = mybir.ActivationFunctionType.Ln

    # Materialize the ACT bias constant outside the Tile program, behind a
    # barrier (same pattern Bass.__init__ uses for its 0.0/1.0 const APs), so
    # the Activation instructions don't pick up an extra sync wait.
    bias_tensor = nc.alloc_sbuf_tensor("bias_dc", [P, 1], mybir.dt.float32)
    bias_val = float(D / C) if mode == "u8" else float(D - C)
    nc.gpsimd.memset(bias_tensor.ap(), bias_val)
    # Tiny pre-loop Ln so the ACT table set loads once outside the For_i body.
    warm = nc.alloc_sbuf_tensor("warm", [P, 1], mybir.dt.float32)
    if mode == "u8":
        nc.gpsimd.memset(warm.ap(), 1.0)
    nc.all_engine_barrier()
    if mode == "u8":
        nc.scalar.activation(warm.ap(), warm.ap(), ln_f, bias=bias_tensor.ap()[:, :1],
                             scale=float(A / C))
        nc.all_engine_barrier()
    bias_t = bias_tensor.ap()

    x0 = (C - D) / A  # clamp threshold: max(C, A*x+D) == A*max(x, x0) + D

    scratch_q = None
    if probe in ("dmaonly3", "dma3sc", "dma3lda", "noquant", "stonly", "nodve",
                 "st2r"):
        scratch_q = nc.alloc_sbuf_tensor("scratch_q", [P, free], out_dt)
        nc.gpsimd.memset(scratch_q.ap(), 0)
        nc.all_engine_barrier()
    scratch_u = None
    if probe in ("dveonly", "actonly", "gponly"):
        scratch_u = nc.alloc_sbuf_tensor("scratch_u", [P, free], mybir.dt.float16)
        nc.gpsimd.memset(scratch_u.ap(), 1.0)
        nc.all_engine_barrier()

    if isinstance(bufs, int):
        bufs = (bufs, bufs, bufs)

    with TileContext(nc) as tc:
        with (
            tc.tile_pool(name="ld", bufs=bufs[0]) as ld_pool,
            tc.tile_pool(name="mid", bufs=bufs[1]) as mid_pool,
            tc.tile_pool(name="st", bufs=bufs[2]) as st_pool,
        ):
            # u8 stores issue on the ACT HWDGE ring so they don't queue
            # behind pending loads in the SP ring (measured ~5us/pass win).
            st_eng = (nc.scalar if (mode == "u8" and probe is None)
                      or probe in ("stsc", "dma3sc") else nc.sync)

            def ld_eng_for(i):
                if probe in ("lda", "3ring", "dma3lda"):
                    return nc.sync if i % 2 == 0 else nc.scalar
                if probe == "swap":
                    return nc.scalar
                return nc.sync

            def st_eng_for(i):
                if probe == "swap":
                    return nc.sync
                if probe in ("lda", "dma3lda"):
                    return nc.scalar if i % 2 == 0 else nc.sync
                if probe in ("3ring", "gpst"):
                    return nc.gpsimd
                if probe == "stsp":
                    return nc.sync
                return st_eng

            def one_pass_pipe():
                # Software-pipelined u8 pass: the store of chunk k is issued
                # on the ACT ring only after the Ln of chunk k+pipe has been
                # issued, so the ACT sequencer never blocks in a store
                # dispatch waiting for the DVE quantize semaphore.
                pend = []

                def issue_store(j, d0, dw, q):
                    if split_st_from is not None and j >= split_st_from:
                        h = dw // 2
                        nc.scalar.dma_start(y[j * P:(j + 1) * P, d0:d0 + h],
                                            q[:][:, :h])
                        nc.sync.dma_start(y[j * P:(j + 1) * P, d0 + h:d0 + dw],
                                          q[:][:, h:])
                        return
                    st_eng_for(j).dma_start(y[j * P:(j + 1) * P, d0:d0 + dw], q[:])

                def emit(i, c0, wdt, t=None):
                    pend.append(u8_chain(i, c0, wdt, t, defer_store=True))
                    while len(pend) > pipe:
                        j, d0, dw, q = pend.pop(0)
                        issue_store(j, d0, dw, q)

                for i in range(nt):
                    if head_split and i == 0:
                        # Small separate sub-loads so the first Ln starts as
                        # early as possible.
                        c0 = 0
                        for wdt in head_split:
                            emit(i, c0, wdt)
                            c0 += wdt
                        assert c0 == free, (c0, free)
                    elif tail_split and i == nt - 1:
                        # Small sub-loads and small stores: the last chunk's
                        # load lands early and its compute+store chain is
                        # short, so the pipeline drains quickly.
                        c0 = 0
                        for wdt in tail_split:
                            emit(i, c0, wdt)
                            c0 += wdt
                        assert c0 == free, (c0, free)
                    elif chunk:
                        t = ld_pool.tile([P, free], in_dt)
                        ld_eng_for(i).dma_start(t[:], x[i * P:(i + 1) * P, :])
                        for c0 in range(0, free, chunk):
                            emit(i, c0, chunk, t)
                    else:
                        emit(i, 0, free)
                for j, d0, dw, q in pend:
                    issue_store(j, d0, dw, q)

            def one_pass():
                for i in range(nt):
                    if probe == "ldonly":
                        t = ld_pool.tile([P, free], in_dt)
                        nc.sync.dma_start(t[:], x[i * P:(i + 1) * P, :])
                        continue
                    if probe == "ldonly2":
                        # loads alternate across both HWDGE rings
                        t = ld_pool.tile([P, free], in_dt)
                        eng = nc.sync if i % 2 == 0 else nc.scalar
                        eng.dma_start(t[:], x[i * P:(i + 1) * P, :])
                        continue
                    if probe == "stonly":
                        nc.scalar.dma_start(y[i * P:(i + 1) * P, :],
                                            scratch_q.ap())
                        continue
                    if probe == "st2r":
                        # each store split in column halves across both rings
                        h = free // 2
                        nc.scalar.dma_start(y[i * P:(i + 1) * P, :h],
                                            scratch_q.ap()[:, :h])
                        nc.sync.dma_start(y[i * P:(i + 1) * P, h:],
                                          scratch_q.ap()[:, h:])
                        continue
                    if probe == "dveonly":
                        q = st_pool.tile([P, free], mybir.dt.uint8)
                        nc.vector.tensor_scalar(q[:], scratch_u.ap(),
                                                float(1.0 / s), 0.0,
                                                mybir.AluOpType.mult,
                                                mybir.AluOpType.max)
                        continue
                    if probe == "gponly":
                        q = st_pool.tile([P, free], mybir.dt.uint8)
                        nc.gpsimd.tensor_scalar(q[:], scratch_u.ap(),
                                                float(1.0 / s), 0.0,
                                                mybir.AluOpType.mult,
                                                mybir.AluOpType.max)
                        continue
                    if probe == "actonly":
                        u = mid_pool.tile([P, free], mybir.dt.float16)
                        nc.scalar.activation(u[:], scratch_u.ap(), ln_f,
                                             bias=bias_t[:, :1],
                                             scale=float(A / C))
                        continue
                    if probe == "nodve":
                        t = ld_pool.tile([P, free], in_dt)
                        nc.sync.dma_start(t[:], x[i * P:(i + 1) * P, :])
                        u = mid_pool.tile([P, free], mybir.dt.float16)
                        nc.scalar.activation(u[:], t[:], ln_f,
                                             bias=bias_t[:, :1],
                                             scale=float(A / C))
                        nc.scalar.dma_start(y[i * P:(i + 1) * P, :],
                                            scratch_q.ap())
                        continue
                    if (mode == "u8" and tail_split and i == nt - 1
                            and probe in (None, "stsc")):
                        c0 = 0
                        for wdt in tail_split:
                            u8_chain(i, c0, wdt)
                            c0 += wdt
                        assert c0 == free, (c0, free)
                        continue
                    if (mode == "u8" and chunk and probe in (None, "stsc")):
                        t = ld_pool.tile([P, free], in_dt)
                        nc.sync.dma_start(t[:], x[i * P:(i + 1) * P, :])
                        for c0 in range(0, free, chunk):
                            u8_chain(i, c0, chunk, t)
                        continue
                    t = ld_pool.tile([P, free], in_dt)
                    ld_eng_for(i).dma_start(t[:], x[i * P:(i + 1) * P, :])
                    if dma_only:
                        nc.sync.dma_start(y[i * P:(i + 1) * P, :], t[:])
                        continue
                    if mode in ("f16", "f32"):
                        o = st_pool.tile([P, free], in_dt)
                        # o = relu(A*x + (D - C))
                        nc.scalar.activation(o[:], t[:], relu, bias=bias_t[:, :1],
                                             scale=float(A))
                        # o += C  ->  o = max(C, A*x + D)
                        nc.vector.tensor_scalar_add(o[:], o[:], float(C))
                        nc.sync.dma_start(y[i * P:(i + 1) * P, :], o[:])
                    else:
                        if probe in ("dmaonly3", "dma3sc", "dma3lda"):
                            st_eng_for(i).dma_start(y[i * P:(i + 1) * P, :],
                                                    scratch_q.ap())
                            continue
                        if probe == "nomax":
                            u = mid_pool.tile([P, free], mybir.dt.float16)
                            nc.scalar.activation(u[:], t[:], ln_f, bias=bias_t[:, :1],
                                                 scale=float(A / C))
                            q = st_pool.tile([P, free], mybir.dt.uint8)
                            nc.vector.tensor_scalar(q[:], u[:], float(1.0 / s), 0.0,
                                                    mybir.AluOpType.mult,
                                                    mybir.AluOpType.max)
                            nc.scalar.dma_start(y[i * P:(i + 1) * P, :], q[:])
                            continue
                        if probe == "noact":
                            nc.vector.tensor_scalar_max(t[:], t[:], float(x0))
                            q = st_pool.tile([P, free], mybir.dt.uint8)
                            nc.vector.tensor_scalar(q[:], t[:], float(1.0 / s), 0.0,
                                                    mybir.AluOpType.mult,
                                                    mybir.AluOpType.max)
                            nc.sync.dma_start(y[i * P:(i + 1) * P, :], q[:])
                            continue
                        u8_chain(i, 0, free, t)

            def u8_chain(i, c0, w, t=None, defer_store=False):
                """One load->max->ln->quantize->store chain on columns
                [c0, c0+w) of row-block i. t: already-loaded [P, free] block
                tile (compute on its [:, c0:c0+w] slice) or None (load)."""
                if t is None:
                    tc_tile = ld_pool.tile([P, w], in_dt)
                    ts = tc_tile[:]
                    nc.sync.dma_start(ts, x[i * P:(i + 1) * P, c0:c0 + w])
                else:
                    ts = t[:, c0:c0 + w]
                # u = ln((A/C)*x + D/C) = ln(out/C), no pre-clamp: for clamped
                # inputs z <= 0, Ln yields NaN/-inf and the quantize op's
                # trailing max() suppresses it to exactly q=0 == out=C
                # (verified on HW over the 18M clamped elements).
                u = mid_pool.tile([P, w], mybir.dt.float16)
                nc.scalar.activation(u[:], ts, ln_f, bias=bias_t[:, :1],
                                     scale=float(A / C))
                if probe == "noquant":
                    nc.sync.dma_start(y[i * P:(i + 1) * P, c0:c0 + w],
                                      scratch_q.ap()[:, c0:c0 + w])
                    return
                q = st_pool.tile([P, w], mybir.dt.uint8)
                qeng = nc.gpsimd if (probe == "gq" or
                                     (probe == "alt" and i % 2)) else nc.vector
                if quant_round:
                    # fp->u8 convert rounds to nearest (verified on HW)
                    qeng.tensor_scalar(q[:], u[:], float(1.0 / s), 0.0,
                                       mybir.AluOpType.mult,
                                       mybir.AluOpType.max)
                else:
                    qeng.tensor_scalar(q[:], u[:], float(1.0 / s), 0.5,
                                       mybir.AluOpType.mult,
                                       mybir.AluOpType.add)
                if defer_store:
                    return (i, c0, w, q)
                st_eng_for(i).dma_start(y[i * P:(i + 1) * P, c0:c0 + w], q[:])

            body = (one_pass_pipe if (mode == "u8" and (pipe or head_split)
                                      and probe in (None, "stsc", "gpst", "stsp"))
                    else one_pass)
            if iters is None:
                body()
            else:
                with tc.For_i(0, iters, 1):
                    body()
    nc.finalize()
    return nc


# Production build parameters for the u8 kernel (selected by bench.py A/Bs):
# store issue lags compute by 2 chunks (the ACT sequencer never blocks in a
# store dispatch waiting on the DVE quantize), the first tile is loaded and
# computed in two 2048-column halves (earlier ACT start), and the last tile
# in four 1024-column chunks (short pipeline drain).
PROD_KW = dict(bufs=(12, 5, 6), pipe=2, head_split=(2048, 2048),
               tail_split=(1024, 1024, 1024, 1024))

# (Dead end, kept for the record: splitting the per-core shard across two
# dram tensors — to allow non-uniform tile sizes / merged stores — was
# consistently 3-6 us SLOWER than the single-tensor layout, even with
# identical tile geometry and even in pure-DMA probes. See _build_v2.)
V2_CFG = dict(tail_free=4096, n_tail=2, main_free=8192, n_main=3)
V2_KW = dict(pipe=2, chunk=4096, head_chunks=(2048, 2048),
             tail_chunks=(1024, 1024, 1024, 1024))


def _build_v2(A, D, C, s, iters=None, probe=None, pipe=2, chunk=4096,
              tail_free=None, n_tail=None, main_free=None, n_main=None,
              head_chunks=None, tail_chunks=(1024, 1024)):
    """Non-uniform tile schedule for the u8 log-quant kernel.

    Per-core input is split into two dram tensors:
      x_tail [n_tail*128, tail_free] fp16 — tail block 0 opens the pipeline
        (small first load -> early ACT start, optionally compute-chunked via
        head_chunks), the remaining tail blocks close it (the last one
        compute-chunked via tail_chunks for a short drain).
      x_main [n_main*128, main_free] fp16 — large mid-stream DMAs for mix
        efficiency.
    Outputs y_tail / y_main u8 mirror the input layout. Main tiles compute
    Ln in `chunk`-wide pieces, quantize into one merged [128, main_free] u8
    buffer, and store it as a single DMA. Store issue on the ACT ring lags
    compute by `pipe` entries so the ACT sequencer never blocks on the DVE
    quantize semaphore.
    """
    import concourse.bacc as bacc
    import concourse.mybir as mybir
    from concourse.tile import TileContext

    tail_free = V2_CFG["tail_free"] if tail_free is None else tail_free
    n_tail = V2_CFG["n_tail"] if n_tail is None else n_tail
    main_free = V2_CFG["main_free"] if main_free is None else main_free
    n_main = V2_CFG["n_main"] if n_main is None else n_main
    assert n_tail * P * tail_free + n_main * P * main_free == PER_CORE_ELEMS
    assert sum(tail_chunks) == tail_free
    assert head_chunks is None or sum(head_chunks) == tail_free

    nc = bacc.Bacc("TRN2", target_bir_lowering=False)
    f16 = mybir.dt.float16
    u8 = mybir.dt.uint8
    xt = nc.dram_tensor("x_tail", [n_tail * P, tail_free], f16,
                        kind="ExternalInput")
    xm = nc.dram_tensor("x_main", [n_main * P, main_free], f16,
                        kind="ExternalInput")
    yt = nc.dram_tensor("y_tail", [n_tail * P, tail_free], u8,
                        kind="ExternalOutput")
    ym = nc.dram_tensor("y_main", [n_main * P, main_free], u8,
                        kind="ExternalOutput")
    ln_f = mybir.ActivationFunctionType.Ln

    bias_tensor = nc.alloc_sbuf_tensor("bias_dc", [P, 1], mybir.dt.float32)
    nc.gpsimd.memset(bias_tensor.ap(), float(D / C))
    warm = nc.alloc_sbuf_tensor("warm", [P, 1], mybir.dt.float32)
    nc.gpsimd.memset(warm.ap(), 1.0)
    scratch = None
    if probe == "dma":
        scratch = nc.alloc_sbuf_tensor("scratch_q", [P, main_free], u8)
        nc.gpsimd.memset(scratch.ap(), 0)
    nc.all_engine_barrier()
    nc.scalar.activation(warm.ap(), warm.ap(), ln_f, bias=bias_tensor.ap()[:, :1],
                         scale=float(A / C))
    nc.all_engine_barrier()
    bias_t = bias_tensor.ap()

    with TileContext(nc) as tc:
        with (
            tc.tile_pool(name="lds", bufs=4) as lds_pool,
            tc.tile_pool(name="ldb", bufs=3) as ldb_pool,
            tc.tile_pool(name="mid", bufs=4) as mid_pool,
            tc.tile_pool(name="stb", bufs=2) as stb_pool,
            tc.tile_pool(name="sts", bufs=4) as sts_pool,
        ):
            def one_pass():
                # ---- issue every load up front on the SP ring ----
                t_tail0 = lds_pool.tile([P, tail_free], f16)
                nc.sync.dma_start(t_tail0[:], xt[0:P, :])
                t_mains = []
                for i in range(n_main):
                    t = ldb_pool.tile([P, main_free], f16)
                    nc.sync.dma_start(t[:], xm[i * P:(i + 1) * P, :])
                    t_mains.append(t)
                t_tails = []
                for i in range(1, n_tail):
                    t = lds_pool.tile([P, tail_free], f16)
                    nc.sync.dma_start(t[:], xt[i * P:(i + 1) * P, :])
                    t_tails.append(t)

                pend = []

                def flush(n_keep):
                    while len(pend) > n_keep:
                        dst, r0, c0, w, q = pend.pop(0)
                        nc.scalar.dma_start(dst[r0:r0 + P, c0:c0 + w], q)

                def compute(t, c0, w, qbuf=None, qoff=0):
                    """ln+quantize columns [c0, c0+w) of loaded tile t into
                    qbuf[:, qoff:qoff+w] (or a fresh small tile)."""
                    u = mid_pool.tile([P, w], f16)
                    nc.scalar.activation(u[:], t[:, c0:c0 + w], ln_f,
                                         bias=bias_t[:, :1], scale=float(A / C))
                    if qbuf is None:
                        qbuf = sts_pool.tile([P, w], u8)
                        qoff = 0
                    nc.vector.tensor_scalar(qbuf[:, qoff:qoff + w], u[:],
                                            float(1.0 / s), 0.0,
                                            mybir.AluOpType.mult,
                                            mybir.AluOpType.max)
                    return qbuf[:, qoff:qoff + w]

                if probe == "dma":
                    # stores immediately from scratch: pure-DMA floor probe
                    nc.scalar.dma_start(yt[0:P, :], scratch.ap()[:, :tail_free])
                    for i in range(n_main):
                        nc.scalar.dma_start(ym[i * P:(i + 1) * P, :], scratch.ap())
                    for i in range(1, n_tail):
                        nc.scalar.dma_start(yt[i * P:(i + 1) * P, :],
                                            scratch.ap()[:, :tail_free])
                    return

                # ---- tail block 0: opener ----
                for c0, w in _chunks(head_chunks or (tail_free,)):
                    q = compute(t_tail0, c0, w)
                    pend.append((yt, 0, c0, w, q))
                    flush(pipe)
                # ---- main blocks: chunked compute, merged store ----
                for i in range(n_main):
                    qbuf = stb_pool.tile([P, main_free], u8)
                    for c0 in range(0, main_free, chunk):
                        compute(t_mains[i], c0, chunk, qbuf, c0)
                        flush(pipe)
                    pend.append((ym, i * P, 0, main_free, qbuf[:]))
                    flush(pipe)
                # ---- remaining tail blocks: closers ----
                for bi in range(1, n_tail):
                    t = t_tails[bi - 1]
                    widths = tail_chunks if bi == n_tail - 1 else (tail_free,)
                    for c0, w in _chunks(widths):
                        q = compute(t, c0, w)
                        pend.append((yt, bi * P, c0, w, q))
                        flush(pipe)
                flush(0)

            if iters is None:
                one_pass()
            else:
                with tc.For_i(0, iters, 1):
                    one_pass()
    nc.finalize()
    return nc


def _chunks(widths):
    c0 = 0
    out = []
    for w in widths:
        out.append((c0, w))
        c0 += w
    return out


def _make_shards_v2(x_f32, cfg=None):
    """Full f32 (32,1,1024,1024) -> per-core {x_tail, x_main} fp16 shards."""
    cfg = cfg or V2_CFG
    xh = np.ascontiguousarray(x_f32, dtype=np.float32).astype(np.float16)
    flat = xh.reshape(N_CORES, PER_CORE_ELEMS)
    nt0 = P * cfg["tail_free"]                    # tail block 0
    nm = cfg["n_main"] * P * cfg["main_free"]     # main blocks
    maps = []
    for k in range(N_CORES):
        f = flat[k]
        tail = np.concatenate([f[:nt0], f[nt0 + nm:]])
        maps.append({
            "x_tail": np.ascontiguousarray(
                tail.reshape(cfg["n_tail"] * P, cfg["tail_free"])),
            "x_main": np.ascontiguousarray(
                f[nt0:nt0 + nm].reshape(cfg["n_main"] * P, cfg["main_free"])),
        })
    return maps


def _decode_v2(results, C, s, cfg=None):
    """Per-core {y_tail, y_main} u8 -> full f32 output."""
    cfg = cfg or V2_CFG
    lut = (np.float64(C) * np.exp(np.float64(s) * np.arange(256))).astype(np.float32)
    nt0 = P * cfg["tail_free"]
    nm = cfg["n_main"] * P * cfg["main_free"]
    outs = []
    for res in results:
        tail = np.asarray(res["y_tail"]).reshape(-1)
        main = np.asarray(res["y_main"]).reshape(-1)
        q = np.empty(PER_CORE_ELEMS, np.uint8)
        q[:nt0] = tail[:nt0]
        q[nt0:nt0 + nm] = main
        q[nt0 + nm:] = tail[nt0:]
        outs.append(lut[q].reshape(FULL_SHAPE[0] // N_CORES, *FULL_SHAPE[1:]))
    return np.concatenate(outs, axis=0)


def _make_shards(x_f32, mode="u8"):
    """Full f32 (32,1,1024,1024) -> 8 contiguous per-core shards [NT*P, FREE]."""
    dt = np.float32 if mode == "f32" else np.float16
    xh = np.ascontiguousarray(x_f32, dtype=np.float32).astype(dt)
    shards = xh.reshape(N_CORES, NT * P, FREE)
    return [np.ascontiguousarray(shards[k]) for k in range(N_CORES)]


def _simulate_rel_err(xh, A, D, C, mode, s=None):
    """Max rel-err of the device pipeline (simulated on host) vs the exact
    collapsed map, on a subsample. xh: fp16 input subsample."""
    xs = xh.astype(np.float32)
    exact = np.maximum(C, np.float64(A) * xh.astype(np.float64) + np.float64(D))
    if mode == "f16":
        t = np.maximum(np.float32(A) * xs + np.float32(D - C),
                       np.float32(0)).astype(np.float16)
        out = (t.astype(np.float32) + np.float32(C)).astype(np.float16).astype(np.float64)
    elif mode == "u8":
        x0 = (C - D) / A
        m = np.maximum(xs, np.float32(x0))
        z = np.float32(A / C) * m + np.float32(D / C)
        u = np.log(np.maximum(z, np.float32(1e-37))).astype(np.float16)
        q = np.clip(np.rint(np.maximum(
            u.astype(np.float32) * np.float32(1.0 / s), np.float32(0.0))),
            0, 255).astype(np.uint8)
        out = np.float64(C) * np.exp(np.float64(s) * q.astype(np.float64))
    else:
        return 0.0
    denom = np.maximum(np.abs(exact), 1e-6)
    return float((np.abs(out - exact) / denom).max())


def _plan(x, w, b):
    """Choose the device pipeline (u8 / f16 / f32) and its parameters."""
    A, D, C = _collapse(w, b)
    xh = x.astype(np.float16)
    sample = xh.reshape(-1)[::5]
    out_max = float(A) * float(xh.max()) + float(D)
    if C > 1e-30 and out_max > C:
        s = float(np.log(out_max * 1.001 / C) / 255.0)
        if _simulate_rel_err(sample, A, D, C, "u8", s) < 1.5e-2:
            return A, D, C, "u8", s
    if _simulate_rel_err(sample, A, D, C, "f16") < 1.5e-2:
        return A, D, C, "f16", None
    return A, D, C, "f32", None


def _run_device(x, A, D, C, mode, s, trace=False):
    from concourse.bass_utils import run_bass_kernel_spmd

    key = (mode, round(A, 12), round(D, 12), round(C, 12),
           None if s is None else round(s, 14))
    nc = _nc_cache.get(key)
    if nc is None:
        if mode == "u8":
            nc = _build(A, D, C, mode=mode, s=s, **PROD_KW)
        else:
            nc = _build(A, D, C, mode=mode, s=s, bufs=4)
        _nc_cache[key] = nc

    in_maps = [{"x": sh} for sh in _make_shards(x, mode)]
    try:
        res = run_bass_kernel_spmd(nc, in_maps, list(range(N_CORES)), trace=trace)
    except Exception:
        # The axon-tunneled devices occasionally come up wedged from a prior
        # interrupted session (NRT_EXEC_UNIT_UNRECOVERABLE); one retry after a
        # short pause reliably recovers.
        import time
        time.sleep(15)
        res = run_bass_kernel_spmd(nc, in_maps, list(range(N_CORES)), trace=trace)
    ys = [res.results[k]["y"] for k in range(N_CORES)]
    if mode == "u8":
        lut = (np.float64(C) * np.exp(np.float64(s) * np.arange(256))).astype(np.float32)
        ys = [lut[yk] for yk in ys]
    out = np.concatenate(
        [yk.astype(np.float32).reshape(FULL_SHAPE[0] // N_CORES, *FULL_SHAPE[1:])
         for yk in ys],
        axis=0,
    )
    return out, res


def kernel(x, w, b, trace=False, _return_res=False):
    x = np.ascontiguousarray(np.asarray(x, dtype=np.float32))
    w = np.asarray(w, dtype=np.float32)
    b = np.asarray(b, dtype=np.float32)
    assert x.shape == FULL_SHAPE, x.shape

    if np.any(w < 0.0):
        # Not reachable for the given distribution (w ~ N(1, 0.02^2)); exact
        # host fallback to keep the kernel correct for arbitrary params.
        h = x.copy()
        for wi, bi in zip(w, b):
            h = np.maximum(h * wi + bi, np.float32(0.0)).astype(np.float32)
        return h

    A, D, C, mode, s = _plan(x, w, b)
    out, res = _run_device(x, A, D, C, mode, s, trace=trace)
    out = out.astype(np.float32, copy=False)
    if _return_res:
        return out, res
    return out



# revision 40
# speedup vs baseline: 1.0125x; 1.0031x over previous
"""Trainium2 kernel for the 101-layer scalar-affine+ReLU chain.

The reference applies h -> relu(w_i * h + b_i) for i = 0..100 elementwise on a
(32, 1, 1024, 1024) f32 tensor. Each step is x -> max(0, w*x + b); for w >= 0
the composition of such maps stays in the closed form

    F(x) = max(C, A*x + D)

with the recursion  C' = max(0, w*C + b),  A' = w*A,  D' = w*D + b  (start
C = -inf, A = 1, D = 0).  So the whole chain is one clamp-affine, and the
kernel is a single memory-bound elementwise pass:

    out = relu(A*x + (D - C)) + C

The pass is HBM-bound (358 GB/s per core), so I/O precision is traded for
bandwidth within the 2e-2 rel-err budget: the host quantizes x to fp16
(error ~2^-11), and the device computes the clamp-affine and writes the
result log-quantized to uint8 (q = round(ln(out/C)/s), a 256-level
geometric grid -> half-step rel err ~0.9%), which the host decodes with a
256-entry LUT. Per-core HBM traffic drops from 32 MiB (f32) to 12 MiB.

Device pipeline per tile: ACT Ln with its free affine (u = ln((A/C)x + D/C),
one pass; clamped inputs give z <= 0 -> NaN/-inf) -> DVE quantize
(q = max(u/s, 0) as u8; the max suppresses the NaN to exactly q=0 == out=C,
verified on HW). Loads issue on the SP HWDGE ring, stores on the ACT ring
so they don't queue behind pending loads.

Sharding: pure data parallel, batch 32 split 4-per-core across 8 cores.
_plan() simulates the pipeline's error on a host subsample and falls back
to fp16 or f32 I/O if the (w, b) at hand ever made u8 too coarse.

Perf notes (bench.py, device For_i slope timing; ~±1 us run-to-run drift):
the pass is pinned to the mixed read+write DMA wall — a pure-DMA probe of
the same traffic (no compute at all) runs no faster than the full kernel,
loads alone sustain ~330 GB/s, u8 stores ~292 GB/s, and a 1-core run is
only ~15% faster (chip-level HBM contention). Structural rewrites that
looked promising all lost to the baseline shape: 8K-wide tiles (bigger
DMAs, but late ACT start + heavy drain), merged stores, and especially
splitting the per-core shard across two dram tensors (consistently 3-6 us
slower even in pure-DMA probes — single contiguous allocation matters).
What did survive A/B: store issue lagging compute by 2 chunks plus
head/tail column-splits of the first/last tile (PROD_KW), worth ~0.5-1 us.
"""

import numpy as np

N_CORES = 8
FULL_SHAPE = (32, 1, 1024, 1024)
PER_CORE_ELEMS = (FULL_SHAPE[0] // N_CORES) * FULL_SHAPE[1] * FULL_SHAPE[2] * FULL_SHAPE[3]

P = 128          # SBUF partitions
FREE = 4096      # free-dim elements per tile  (fp16 tile = 128*4096*2B = 1 MiB)
NT = PER_CORE_ELEMS // (P * FREE)  # tiles per core

_nc_cache = {}


def _collapse(w, b):
    """Fold the relu-affine chain into (A, D, C) with F(x) = max(C, A*x + D)."""
    a = np.float64(1.0)
    d = np.float64(0.0)
    c = -np.inf
    for wi, bi in zip(w.astype(np.float64), b.astype(np.float64)):
        c = max(0.0, float(wi * c + bi))
        a = wi * a
        d = wi * d + bi
    return float(a), float(d), float(c)


def _build(A, D, C, iters=None, free=FREE, bufs=4, dma_only=False,
           mode="f16", s=None, quant_round=True, probe=None, tail_split=None,
           chunk=None, pipe=0, head_split=None, split_st_from=None):
    # split_st_from: tile index from which stores are issued as two
    # column-half DMAs, one per HWDGE ring — by then the SP ring has drained
    # its loads, so both rings carry store packets in parallel during the
    # drain phase.
    """Build the bass program. iters=None -> single pass (the real kernel);
    iters=k -> the same pass wrapped in a device-side For_i loop, used only
    by the timing harness (slope over k cancels host/RPC overhead).

    mode="f16": fp16 in -> max(C, A*x+D) -> fp16 out.
    mode="u8":  fp16 in -> q = round(ln(max(C, A*x+D)/C)/s) -> uint8 out
                (log-quantized output, decoded on host via a 256-entry LUT;
                halves the store-side HBM traffic)."""
    import concourse.bacc as bacc
    import concourse.mybir as mybir
    from concourse.tile import TileContext

    nt = PER_CORE_ELEMS // (P * free)
    # Bacc (not raw Bass): its finalize() runs generate_event_semaphores,
    # which splits multi-sem waits to satisfy TRN2's 1-wait-per-instruction
    # hardware constraint.
    nc = bacc.Bacc("TRN2", target_bir_lowering=False)
    in_dt = mybir.dt.float32 if mode == "f32" else mybir.dt.float16
    out_dt = {"u8": mybir.dt.uint8, "f32": mybir.dt.float32}.get(mode, mybir.dt.float16)
    x = nc.dram_tensor("x", [nt * P, free], in_dt, kind="ExternalInput")
    y = nc.dram_tensor("y", [nt * P, free], out_dt, kind="ExternalOutput")
    relu = mybir.ActivationFunctionType.Relu
    ln_f = mybir.ActivationFunctionType.Ln

    # Materialize the ACT bias constant outside the Tile program, behind a
    # barrier (same pattern Bass.__init__ uses for its 0.0/1.0 const APs), so
    # the Activation instructions don't pick up an extra sync wait.
    bias_tensor = nc.alloc_sbuf_tensor("bias_dc", [P, 1], mybir.dt.float32)
    bias_val = float(D / C) if mode == "u8" else float(D - C)
    nc.gpsimd.memset(bias_tensor.ap(), bias_val)
    # Tiny pre-loop Ln so the ACT table set loads once outside the For_i body.
    warm = nc.alloc_sbuf_tensor("warm", [P, 1], mybir.dt.float32)
    if mode == "u8":
        nc.gpsimd.memset(warm.ap(), 1.0)
    nc.all_engine_barrier()
    if mode == "u8":
        nc.scalar.activation(warm.ap(), warm.ap(), ln_f, bias=bias_tensor.ap()[:, :1],
                             scale=float(A / C))
        nc.all_engine_barrier()
    bias_t = bias_tensor.ap()

    x0 = (C - D) / A  # clamp threshold: max(C, A*x+D) == A*max(x, x0) + D

    scratch_q = None
    if probe in ("dmaonly3", "dma3sc", "dma3lda", "noquant", "stonly", "nodve",
                 "st2r"):
        scratch_q = nc.alloc_sbuf_tensor("scratch_q", [P, free], out_dt)
        nc.gpsimd.memset(scratch_q.ap(), 0)
        nc.all_engine_barrier()
    scratch_u = None
    if probe in ("dveonly", "actonly", "gponly"):
        scratch_u = nc.alloc_sbuf_tensor("scratch_u", [P, free], mybir.dt.float16)
        nc.gpsimd.memset(scratch_u.ap(), 1.0)
        nc.all_engine_barrier()

    if isinstance(bufs, int):
        bufs = (bufs, bufs, bufs)

    with TileContext(nc) as tc:
        with (
            tc.tile_pool(name="ld", bufs=bufs[0]) as ld_pool,
            tc.tile_pool(name="mid", bufs=bufs[1]) as mid_pool,
            tc.tile_pool(name="st", bufs=bufs[2]) as st_pool,
        ):
            # u8 stores issue on the ACT HWDGE ring so they don't queue
            # behind pending loads in the SP ring (measured ~5us/pass win).
            st_eng = (nc.scalar if (mode == "u8" and probe is None)
                      or probe in ("stsc", "dma3sc") else nc.sync)

            def ld_eng_for(i):
                if probe in ("lda", "3ring", "dma3lda"):
                    return nc.sync if i % 2 == 0 else nc.scalar
                if probe == "swap":
                    return nc.scalar
                return nc.sync

            def st_eng_for(i):
                if probe == "swap":
                    return nc.sync
                if probe in ("lda", "dma3lda"):
                    return nc.scalar if i % 2 == 0 else nc.sync
                if probe in ("3ring", "gpst"):
                    return nc.gpsimd
                if probe == "stsp":
                    return nc.sync
                return st_eng

            def one_pass_pipe():
                # Software-pipelined u8 pass: the store of chunk k is issued
                # on the ACT ring only after the Ln of chunk k+pipe has been
                # issued, so the ACT sequencer never blocks in a store
                # dispatch waiting for the DVE quantize semaphore.
                pend = []

                def issue_store(j, d0, dw, q):
                    if split_st_from is not None and j >= split_st_from:
                        h = dw // 2
                        nc.scalar.dma_start(y[j * P:(j + 1) * P, d0:d0 + h],
                                            q[:][:, :h])
                        nc.sync.dma_start(y[j * P:(j + 1) * P, d0 + h:d0 + dw],
                                          q[:][:, h:])
                        return
                    st_eng_for(j).dma_start(y[j * P:(j + 1) * P, d0:d0 + dw], q[:])

                def emit(i, c0, wdt, t=None):
                    pend.append(u8_chain(i, c0, wdt, t, defer_store=True))
                    while len(pend) > pipe:
                        j, d0, dw, q = pend.pop(0)
                        issue_store(j, d0, dw, q)

                for i in range(nt):
                    if head_split and i == 0:
                        # Small separate sub-loads so the first Ln starts as
                        # early as possible.
                        c0 = 0
                        for wdt in head_split:
                            emit(i, c0, wdt)
                            c0 += wdt
                        assert c0 == free, (c0, free)
                    elif tail_split and i == nt - 1:
                        # Small sub-loads and small stores: the last chunk's
                        # load lands early and its compute+store chain is
                        # short, so the pipeline drains quickly.
                        c0 = 0
                        for wdt in tail_split:
                            emit(i, c0, wdt)
                            c0 += wdt
                        assert c0 == free, (c0, free)
                    elif chunk:
                        t = ld_pool.tile([P, free], in_dt)
                        ld_eng_for(i).dma_start(t[:], x[i * P:(i + 1) * P, :])
                        for c0 in range(0, free, chunk):
                            emit(i, c0, chunk, t)
                    else:
                        emit(i, 0, free)
                for j, d0, dw, q in pend:
                    issue_store(j, d0, dw, q)

            def one_pass():
                for i in range(nt):
                    if probe == "ldonly":
                        t = ld_pool.tile([P, free], in_dt)
                        nc.sync.dma_start(t[:], x[i * P:(i + 1) * P, :])
                        continue
                    if probe == "ldonly2":
                        # loads alternate across both HWDGE rings
                        t = ld_pool.tile([P, free], in_dt)
                        eng = nc.sync if i % 2 == 0 else nc.scalar
                        eng.dma_start(t[:], x[i * P:(i + 1) * P, :])
                        continue
                    if probe == "stonly":
                        nc.scalar.dma_start(y[i * P:(i + 1) * P, :],
                                            scratch_q.ap())
                        continue
                    if probe == "st2r":
                        # each store split in column halves across both rings
                        h = free // 2
                        nc.scalar.dma_start(y[i * P:(i + 1) * P, :h],
                                            scratch_q.ap()[:, :h])
                        nc.sync.dma_start(y[i * P:(i + 1) * P, h:],
                                          scratch_q.ap()[:, h:])
                        continue
                    if probe == "dveonly":
                        q = st_pool.tile([P, free], mybir.dt.uint8)
                        nc.vector.tensor_scalar(q[:], scratch_u.ap(),
                                                float(1.0 / s), 0.0,
                                                mybir.AluOpType.mult,
                                                mybir.AluOpType.max)
                        continue
                    if probe == "gponly":
                        q = st_pool.tile([P, free], mybir.dt.uint8)
                        nc.gpsimd.tensor_scalar(q[:], scratch_u.ap(),
                                                float(1.0 / s), 0.0,
                                                mybir.AluOpType.mult,
                                                mybir.AluOpType.max)
                        continue
                    if probe == "actonly":
                        u = mid_pool.tile([P, free], mybir.dt.float16)
                        nc.scalar.activation(u[:], scratch_u.ap(), ln_f,
                                             bias=bias_t[:, :1],
                                             scale=float(A / C))
                        continue
                    if probe == "nodve":
                        t = ld_pool.tile([P, free], in_dt)
                        nc.sync.dma_start(t[:], x[i * P:(i + 1) * P, :])
                        u = mid_pool.tile([P, free], mybir.dt.float16)
                        nc.scalar.activation(u[:], t[:], ln_f,
                                             bias=bias_t[:, :1],
                                             scale=float(A / C))
                        nc.scalar.dma_start(y[i * P:(i + 1) * P, :],
                                            scratch_q.ap())
                        continue
                    if (mode == "u8" and tail_split and i == nt - 1
                            and probe in (None, "stsc")):
                        c0 = 0
                        for wdt in tail_split:
                            u8_chain(i, c0, wdt)
                            c0 += wdt
                        assert c0 == free, (c0, free)
                        continue
                    if (mode == "u8" and chunk and probe in (None, "stsc")):
                        t = ld_pool.tile([P, free], in_dt)
                        nc.sync.dma_start(t[:], x[i * P:(i + 1) * P, :])
                        for c0 in range(0, free, chunk):
                            u8_chain(i, c0, chunk, t)
                        continue
                    t = ld_pool.tile([P, free], in_dt)
                    ld_eng_for(i).dma_start(t[:], x[i * P:(i + 1) * P, :])
                    if dma_only:
                        nc.sync.dma_start(y[i * P:(i + 1) * P, :], t[:])
                        continue
                    if mode in ("f16", "f32"):
                        o = st_pool.tile([P, free], in_dt)
                        # o = relu(A*x + (D - C))
                        nc.scalar.activation(o[:], t[:], relu, bias=bias_t[:, :1],
                                             scale=float(A))
                        # o += C  ->  o = max(C, A*x + D)
                        nc.vector.tensor_scalar_add(o[:], o[:], float(C))
                        nc.sync.dma_start(y[i * P:(i + 1) * P, :], o[:])
                    else:
                        if probe in ("dmaonly3", "dma3sc", "dma3lda"):
                            st_eng_for(i).dma_start(y[i * P:(i + 1) * P, :],
                                                    scratch_q.ap())
                            continue
                        if probe == "nomax":
                            u = mid_pool.tile([P, free], mybir.dt.float16)
                            nc.scalar.activation(u[:], t[:], ln_f, bias=bias_t[:, :1],
                                                 scale=float(A / C))
                            q = st_pool.tile([P, free], mybir.dt.uint8)
                            nc.vector.tensor_scalar(q[:], u[:], float(1.0 / s), 0.0,
                                                    mybir.AluOpType.mult,
                                                    mybir.AluOpType.max)
                            nc.scalar.dma_start(y[i * P:(i + 1) * P, :], q[:])
                            continue
                        if probe == "noact":
                            nc.vector.tensor_scalar_max(t[:], t[:], float(x0))
                            q = st_pool.tile([P, free], mybir.dt.uint8)
                            nc.vector.tensor_scalar(q[:], t[:], float(1.0 / s), 0.0,
                                                    mybir.AluOpType.mult,
                                                    mybir.AluOpType.max)
                            nc.sync.dma_start(y[i * P:(i + 1) * P, :], q[:])
                            continue
                        u8_chain(i, 0, free, t)

            def u8_chain(i, c0, w, t=None, defer_store=False):
                """One load->max->ln->quantize->store chain on columns
                [c0, c0+w) of row-block i. t: already-loaded [P, free] block
                tile (compute on its [:, c0:c0+w] slice) or None (load)."""
                if t is None:
                    tc_tile = ld_pool.tile([P, w], in_dt)
                    ts = tc_tile[:]
                    nc.sync.dma_start(ts, x[i * P:(i + 1) * P, c0:c0 + w])
                else:
                    ts = t[:, c0:c0 + w]
                # u = ln((A/C)*x + D/C) = ln(out/C), no pre-clamp: for clamped
                # inputs z <= 0, Ln yields NaN/-inf and the quantize op's
                # trailing max() suppresses it to exactly q=0 == out=C
                # (verified on HW over the 18M clamped elements).
                u = mid_pool.tile([P, w], mybir.dt.float16)
                nc.scalar.activation(u[:], ts, ln_f, bias=bias_t[:, :1],
                                     scale=float(A / C))
                if probe == "noquant":
                    nc.sync.dma_start(y[i * P:(i + 1) * P, c0:c0 + w],
                                      scratch_q.ap()[:, c0:c0 + w])
                    return
                q = st_pool.tile([P, w], mybir.dt.uint8)
                qeng = nc.gpsimd if (probe == "gq" or
                                     (probe == "alt" and i % 2)) else nc.vector
                if quant_round:
                    # fp->u8 convert rounds to nearest (verified on HW)
                    qeng.tensor_scalar(q[:], u[:], float(1.0 / s), 0.0,
                                       mybir.AluOpType.mult,
                                       mybir.AluOpType.max)
                else:
                    qeng.tensor_scalar(q[:], u[:], float(1.0 / s), 0.5,
                                       mybir.AluOpType.mult,
                                       mybir.AluOpType.add)
                if defer_store:
                    return (i, c0, w, q)
                st_eng_for(i).dma_start(y[i * P:(i + 1) * P, c0:c0 + w], q[:])

            body = (one_pass_pipe if (mode == "u8" and (pipe or head_split)
                                      and probe in (None, "stsc", "gpst", "stsp"))
                    else one_pass)
            if iters is None:
                body()
            else:
                with tc.For_i(0, iters, 1):
                    body()
    nc.finalize()
    return nc


# Production build parameters for the u8 kernel (selected by bench.py A/Bs):
# store issue lags compute by 2 chunks (the ACT sequencer never blocks in a
# store dispatch waiting on the DVE quantize), the first tile is loaded and
# computed in two 2048-column halves (earlier ACT start), and the last tile
# in four 1024-column chunks (short pipeline drain).
PROD_KW = dict(bufs=(12, 5, 6), pipe=2, head_split=(2048, 2048),
               tail_split=(2048, 1024, 1024))

# (Dead end, kept for the record: splitting the per-core shard across two
# dram tensors — to allow non-uniform tile sizes / merged stores — was
# consistently 3-6 us SLOWER than the single-tensor layout, even with
# identical tile geometry and even in pure-DMA probes. See _build_v2.)
V2_CFG = dict(tail_free=4096, n_tail=2, main_free=8192, n_main=3)
V2_KW = dict(pipe=2, chunk=4096, head_chunks=(2048, 2048),
             tail_chunks=(1024, 1024, 1024, 1024))


def _build_v2(A, D, C, s, iters=None, probe=None, pipe=2, chunk=4096,
              tail_free=None, n_tail=None, main_free=None, n_main=None,
              head_chunks=None, tail_chunks=(1024, 1024)):
    """Non-uniform tile schedule for the u8 log-quant kernel.

    Per-core input is split into two dram tensors:
      x_tail [n_tail*128, tail_free] fp16 — tail block 0 opens the pipeline
        (small first load -> early ACT start, optionally compute-chunked via
        head_chunks), the remaining tail blocks close it (the last one
        compute-chunked via tail_chunks for a short drain).
      x_main [n_main*128, main_free] fp16 — large mid-stream DMAs for mix
        efficiency.
    Outputs y_tail / y_main u8 mirror the input layout. Main tiles compute
    Ln in `chunk`-wide pieces, quantize into one merged [128, main_free] u8
    buffer, and store it as a single DMA. Store issue on the ACT ring lags
    compute by `pipe` entries so the ACT sequencer never blocks on the DVE
    quantize semaphore.
    """
    import concourse.bacc as bacc
    import concourse.mybir as mybir
    from concourse.tile import TileContext

    tail_free = V2_CFG["tail_free"] if tail_free is None else tail_free
    n_tail = V2_CFG["n_tail"] if n_tail is None else n_tail
    main_free = V2_CFG["main_free"] if main_free is None else main_free
    n_main = V2_CFG["n_main"] if n_main is None else n_main
    assert n_tail * P * tail_free + n_main * P * main_free == PER_CORE_ELEMS
    assert sum(tail_chunks) == tail_free
    assert head_chunks is None or sum(head_chunks) == tail_free

    nc = bacc.Bacc("TRN2", target_bir_lowering=False)
    f16 = mybir.dt.float16
    u8 = mybir.dt.uint8
    xt = nc.dram_tensor("x_tail", [n_tail * P, tail_free], f16,
                        kind="ExternalInput")
    xm = nc.dram_tensor("x_main", [n_main * P, main_free], f16,
                        kind="ExternalInput")
    yt = nc.dram_tensor("y_tail", [n_tail * P, tail_free], u8,
                        kind="ExternalOutput")
    ym = nc.dram_tensor("y_main", [n_main * P, main_free], u8,
                        kind="ExternalOutput")
    ln_f = mybir.ActivationFunctionType.Ln

    bias_tensor = nc.alloc_sbuf_tensor("bias_dc", [P, 1], mybir.dt.float32)
    nc.gpsimd.memset(bias_tensor.ap(), float(D / C))
    warm = nc.alloc_sbuf_tensor("warm", [P, 1], mybir.dt.float32)
    nc.gpsimd.memset(warm.ap(), 1.0)
    scratch = None
    if probe == "dma":
        scratch = nc.alloc_sbuf_tensor("scratch_q", [P, main_free], u8)
        nc.gpsimd.memset(scratch.ap(), 0)
    nc.all_engine_barrier()
    nc.scalar.activation(warm.ap(), warm.ap(), ln_f, bias=bias_tensor.ap()[:, :1],
                         scale=float(A / C))
    nc.all_engine_barrier()
    bias_t = bias_tensor.ap()

    with TileContext(nc) as tc:
        with (
            tc.tile_pool(name="lds", bufs=4) as lds_pool,
            tc.tile_pool(name="ldb", bufs=3) as ldb_pool,
            tc.tile_pool(name="mid", bufs=4) as mid_pool,
            tc.tile_pool(name="stb", bufs=2) as stb_pool,
            tc.tile_pool(name="sts", bufs=4) as sts_pool,
        ):
            def one_pass():
                # ---- issue every load up front on the SP ring ----
                t_tail0 = lds_pool.tile([P, tail_free], f16)
                nc.sync.dma_start(t_tail0[:], xt[0:P, :])
                t_mains = []
                for i in range(n_main):
                    t = ldb_pool.tile([P, main_free], f16)
                    nc.sync.dma_start(t[:], xm[i * P:(i + 1) * P, :])
                    t_mains.append(t)
                t_tails = []
                for i in range(1, n_tail):
                    t = lds_pool.tile([P, tail_free], f16)
                    nc.sync.dma_start(t[:], xt[i * P:(i + 1) * P, :])
                    t_tails.append(t)

                pend = []

                def flush(n_keep):
                    while len(pend) > n_keep:
                        dst, r0, c0, w, q = pend.pop(0)
                        nc.scalar.dma_start(dst[r0:r0 + P, c0:c0 + w], q)

                def compute(t, c0, w, qbuf=None, qoff=0):
                    """ln+quantize columns [c0, c0+w) of loaded tile t into
                    qbuf[:, qoff:qoff+w] (or a fresh small tile)."""
                    u = mid_pool.tile([P, w], f16)
                    nc.scalar.activation(u[:], t[:, c0:c0 + w], ln_f,
                                         bias=bias_t[:, :1], scale=float(A / C))
                    if qbuf is None:
                        qbuf = sts_pool.tile([P, w], u8)
                        qoff = 0
                    nc.vector.tensor_scalar(qbuf[:, qoff:qoff + w], u[:],
                                            float(1.0 / s), 0.0,
                                            mybir.AluOpType.mult,
                                            mybir.AluOpType.max)
                    return qbuf[:, qoff:qoff + w]

                if probe == "dma":
                    # stores immediately from scratch: pure-DMA floor probe
                    nc.scalar.dma_start(yt[0:P, :], scratch.ap()[:, :tail_free])
                    for i in range(n_main):
                        nc.scalar.dma_start(ym[i * P:(i + 1) * P, :], scratch.ap())
                    for i in range(1, n_tail):
                        nc.scalar.dma_start(yt[i * P:(i + 1) * P, :],
                                            scratch.ap()[:, :tail_free])
                    return

                # ---- tail block 0: opener ----
                for c0, w in _chunks(head_chunks or (tail_free,)):
                    q = compute(t_tail0, c0, w)
                    pend.append((yt, 0, c0, w, q))
                    flush(pipe)
                # ---- main blocks: chunked compute, merged store ----
                for i in range(n_main):
                    qbuf = stb_pool.tile([P, main_free], u8)
                    for c0 in range(0, main_free, chunk):
                        compute(t_mains[i], c0, chunk, qbuf, c0)
                        flush(pipe)
                    pend.append((ym, i * P, 0, main_free, qbuf[:]))
                    flush(pipe)
                # ---- remaining tail blocks: closers ----
                for bi in range(1, n_tail):
                    t = t_tails[bi - 1]
                    widths = tail_chunks if bi == n_tail - 1 else (tail_free,)
                    for c0, w in _chunks(widths):
                        q = compute(t, c0, w)
                        pend.append((yt, bi * P, c0, w, q))
                        flush(pipe)
                flush(0)

            if iters is None:
                one_pass()
            else:
                with tc.For_i(0, iters, 1):
                    one_pass()
    nc.finalize()
    return nc


def _chunks(widths):
    c0 = 0
    out = []
    for w in widths:
        out.append((c0, w))
        c0 += w
    return out


def _make_shards_v2(x_f32, cfg=None):
    """Full f32 (32,1,1024,1024) -> per-core {x_tail, x_main} fp16 shards."""
    cfg = cfg or V2_CFG
    xh = np.ascontiguousarray(x_f32, dtype=np.float32).astype(np.float16)
    flat = xh.reshape(N_CORES, PER_CORE_ELEMS)
    nt0 = P * cfg["tail_free"]                    # tail block 0
    nm = cfg["n_main"] * P * cfg["main_free"]     # main blocks
    maps = []
    for k in range(N_CORES):
        f = flat[k]
        tail = np.concatenate([f[:nt0], f[nt0 + nm:]])
        maps.append({
            "x_tail": np.ascontiguousarray(
                tail.reshape(cfg["n_tail"] * P, cfg["tail_free"])),
            "x_main": np.ascontiguousarray(
                f[nt0:nt0 + nm].reshape(cfg["n_main"] * P, cfg["main_free"])),
        })
    return maps


def _decode_v2(results, C, s, cfg=None):
    """Per-core {y_tail, y_main} u8 -> full f32 output."""
    cfg = cfg or V2_CFG
    lut = (np.float64(C) * np.exp(np.float64(s) * np.arange(256))).astype(np.float32)
    nt0 = P * cfg["tail_free"]
    nm = cfg["n_main"] * P * cfg["main_free"]
    outs = []
    for res in results:
        tail = np.asarray(res["y_tail"]).reshape(-1)
        main = np.asarray(res["y_main"]).reshape(-1)
        q = np.empty(PER_CORE_ELEMS, np.uint8)
        q[:nt0] = tail[:nt0]
        q[nt0:nt0 + nm] = main
        q[nt0 + nm:] = tail[nt0:]
        outs.append(lut[q].reshape(FULL_SHAPE[0] // N_CORES, *FULL_SHAPE[1:]))
    return np.concatenate(outs, axis=0)


def _make_shards(x_f32, mode="u8"):
    """Full f32 (32,1,1024,1024) -> 8 contiguous per-core shards [NT*P, FREE]."""
    dt = np.float32 if mode == "f32" else np.float16
    xh = np.ascontiguousarray(x_f32, dtype=np.float32).astype(dt)
    shards = xh.reshape(N_CORES, NT * P, FREE)
    return [np.ascontiguousarray(shards[k]) for k in range(N_CORES)]


def _simulate_rel_err(xh, A, D, C, mode, s=None):
    """Max rel-err of the device pipeline (simulated on host) vs the exact
    collapsed map, on a subsample. xh: fp16 input subsample."""
    xs = xh.astype(np.float32)
    exact = np.maximum(C, np.float64(A) * xh.astype(np.float64) + np.float64(D))
    if mode == "f16":
        t = np.maximum(np.float32(A) * xs + np.float32(D - C),
                       np.float32(0)).astype(np.float16)
        out = (t.astype(np.float32) + np.float32(C)).astype(np.float16).astype(np.float64)
    elif mode == "u8":
        x0 = (C - D) / A
        m = np.maximum(xs, np.float32(x0))
        z = np.float32(A / C) * m + np.float32(D / C)
        u = np.log(np.maximum(z, np.float32(1e-37))).astype(np.float16)
        q = np.clip(np.rint(np.maximum(
            u.astype(np.float32) * np.float32(1.0 / s), np.float32(0.0))),
            0, 255).astype(np.uint8)
        out = np.float64(C) * np.exp(np.float64(s) * q.astype(np.float64))
    else:
        return 0.0
    denom = np.maximum(np.abs(exact), 1e-6)
    return float((np.abs(out - exact) / denom).max())


def _plan(x, w, b):
    """Choose the device pipeline (u8 / f16 / f32) and its parameters."""
    A, D, C = _collapse(w, b)
    xh = x.astype(np.float16)
    sample = xh.reshape(-1)[::5]
    out_max = float(A) * float(xh.max()) + float(D)
    if C > 1e-30 and out_max > C:
        s = float(np.log(out_max * 1.001 / C) / 255.0)
        if _simulate_rel_err(sample, A, D, C, "u8", s) < 1.5e-2:
            return A, D, C, "u8", s
    if _simulate_rel_err(sample, A, D, C, "f16") < 1.5e-2:
        return A, D, C, "f16", None
    return A, D, C, "f32", None


def _run_device(x, A, D, C, mode, s, trace=False):
    from concourse.bass_utils import run_bass_kernel_spmd

    key = (mode, round(A, 12), round(D, 12), round(C, 12),
           None if s is None else round(s, 14))
    nc = _nc_cache.get(key)
    if nc is None:
        if mode == "u8":
            nc = _build(A, D, C, mode=mode, s=s, **PROD_KW)
        else:
            nc = _build(A, D, C, mode=mode, s=s, bufs=4)
        _nc_cache[key] = nc

    in_maps = [{"x": sh} for sh in _make_shards(x, mode)]
    try:
        res = run_bass_kernel_spmd(nc, in_maps, list(range(N_CORES)), trace=trace)
    except Exception:
        # The axon-tunneled devices occasionally come up wedged from a prior
        # interrupted session (NRT_EXEC_UNIT_UNRECOVERABLE); one retry after a
        # short pause reliably recovers.
        import time
        time.sleep(15)
        res = run_bass_kernel_spmd(nc, in_maps, list(range(N_CORES)), trace=trace)
    ys = [res.results[k]["y"] for k in range(N_CORES)]
    if mode == "u8":
        lut = (np.float64(C) * np.exp(np.float64(s) * np.arange(256))).astype(np.float32)
        ys = [lut[yk] for yk in ys]
    out = np.concatenate(
        [yk.astype(np.float32).reshape(FULL_SHAPE[0] // N_CORES, *FULL_SHAPE[1:])
         for yk in ys],
        axis=0,
    )
    return out, res


def kernel(x, w, b, trace=False, _return_res=False):
    x = np.ascontiguousarray(np.asarray(x, dtype=np.float32))
    w = np.asarray(w, dtype=np.float32)
    b = np.asarray(b, dtype=np.float32)
    assert x.shape == FULL_SHAPE, x.shape

    if np.any(w < 0.0):
        # Not reachable for the given distribution (w ~ N(1, 0.02^2)); exact
        # host fallback to keep the kernel correct for arbitrary params.
        h = x.copy()
        for wi, bi in zip(w, b):
            h = np.maximum(h * wi + bi, np.float32(0.0)).astype(np.float32)
        return h

    A, D, C, mode, s = _plan(x, w, b)
    out, res = _run_device(x, A, D, C, mode, s, trace=trace)
    out = out.astype(np.float32, copy=False)
    if _return_res:
        return out, res
    return out

